# revision 11
# baseline (speedup 1.0000x reference)
"""Multi-head attention on 8 Trainium2 NeuronCores.

Problem: x[2, 2048, 1024] -> qkv proj (w_qkv [1024, 3072], 16 heads x 64) ->
softmax attention -> out proj (w_out [1024, 1024] + b_out).

Sharding: core c in 0..7 handles batch b = c // 4 and heads 4*(c%4) .. 4*(c%4)+3.
Each core computes a partial output projection over its 4 heads' slice; the four
cores of each batch group ReduceScatter(add) the partials chunk-by-chunk in bf16
(bias/4 folded in on every core), overlapped with later attention chunks. Core
g of a group ends up with rows [ch*512 + g*128, +128) of each chunk; the host
reassembles the full output from all 8 cores' shards.

Per-core dataflow:
  qkT [512, 2048] = wqk.T @ xT        (fp32r matmuls; Q/K stored bf16,
                                       d-on-partitions layout)
  v   [2048, 260] = xT.T @ wv         (bf16, natural layout, +ones col per head)
  then per query-chunk iq (1024 rows), per head pair (bf16 matmuls, N=1024
  streams; the K=64 head pair shares the PE array via tile rows 0-63/64-127):
    S_T [j, i] = kT.T-slices @ qT
    attn_T = exp(S_T * 0.125)         (ScalarE, scale folded into activation)
    O_T[h] [65, 1024] = v_aug.T @ attn_T  (row 64 = softmax denominator, free)
    o[h] = O_T[0:64] * (1/denom bcast)    (PE K=1 broadcast + DVE multiply)
  y[ch] += o[h].T @ wout[h] (+bias/4)     (fp32r, K=64 per head, PSUM-accum)
  ReduceScatter(y[ch]) in bf16 over the 4-core batch group.
"""

import numpy as np

N = 2048          # sequence length per batch
D = 1024          # model dim
DH = 64           # head dim
HPC = 4           # heads per core
NCORES = 8
GSIZE = 4         # cores per reduce group
SCALE = DH ** -0.5
NCH = N // 512    # output projection / collective chunks

_cached = {}


def _build_nc():
    from contextlib import ExitStack

    import concourse.bacc as bacc
    import concourse.mybir as mybir
    from concourse import tile

    f32 = mybir.dt.float32
    f32r = mybir.dt.float32r
    bf16 = mybir.dt.bfloat16

    nc = bacc.Bacc(num_devices=NCORES)

    xT = nc.declare_dram_parameter("xT", [D, N], f32r, isOutput=False)
    wqk = nc.declare_dram_parameter("wqk", [D, 2 * HPC * DH], f32r, isOutput=False)
    wv = nc.declare_dram_parameter("wv", [D, HPC * DH], f32r, isOutput=False)
    wout = nc.declare_dram_parameter("wout", [HPC * DH, D], f32r, isOutput=False)
    bias = nc.declare_dram_parameter("bias", [1, D], f32r, isOutput=False)
    ones1 = nc.declare_dram_parameter("ones1", [1, 128], f32r, isOutput=False)
    quart = nc.declare_dram_parameter("quart", [1, 128], f32r, isOutput=False)
    # per-core output: NCH shards of 128 rows (this core's ReduceScatter slices)
    y_out = nc.declare_dram_parameter("y", [NCH * 128, D], f32, isOutput=True)

    KB = D // 128           # 8 contraction blocks for the projections
    JB = N // 128           # 16 key blocks
    IQ = N // 1024          # attention query chunks (1024 wide)
    VW = DH + 1             # v columns per head incl. ones column

    with tile.TileContext(nc) as tc:
        ctx = ExitStack()
        with ctx:
            sb = ctx.enter_context(tc.tile_pool(name="sb", bufs=1))
            ps_big = ctx.enter_context(tc.tile_pool(name="ps_big", bufs=2, space="PSUM"))
            ps_o = ctx.enter_context(tc.tile_pool(name="ps_o", bufs=2, space="PSUM"))
            dram = ctx.enter_context(tc.tile_pool(name="dram", bufs=1, space="DRAM"))

            # persistent SBUF residents
            qk_sb = sb.tile([128, 4, N], bf16, tag="qk")
            v_sb = sb.tile([128, JB, HPC * VW], bf16, tag="v")
            o_sb = sb.tile([64, HPC, N], f32r, tag="o")
            wo_sb = sb.tile([64, HPC, D], f32r, tag="wo")
            bias_bc = sb.tile([128, D], f32, tag="bias_bc")
            bias_sb = sb.tile([1, D], f32r, tag="bias")
            ones_sb = sb.tile([1, 128], f32r, tag="ones1")
            quart_sb = sb.tile([1, 128], f32r, tag="quart")

            nc.sync.dma_start(out=bias_sb[:], in_=bias[:, :])
            nc.sync.dma_start(out=ones_sb[:], in_=ones1[:, :])
            nc.sync.dma_start(out=quart_sb[:], in_=quart[:, :])
            for h in range(HPC):
                nc.sync.dma_start(out=wo_sb[:, h, :], in_=wout[h * DH:(h + 1) * DH, :])

            # bias/4 broadcast to 128 partitions (PE K=1 matmul)
            for ch in range(D // 512):
                bps = ps_big.tile([128, 512], f32, tag="big", name=f"bps{ch}")
                nc.tensor.matmul(bps[:], quart_sb[:], bias_sb[:, ch * 512:(ch + 1) * 512],
                                 start=True, stop=True)
                nc.vector.tensor_copy(bias_bc[:, ch * 512:(ch + 1) * 512], bps[:])

            # ---- stage 1: projections (xT + weights live only here) ----
            with ExitStack() as s1:
                sb_x = s1.enter_context(tc.tile_pool(name="sb_x", bufs=1))
                xT_sb = sb_x.tile([128, KB, N], f32r, tag="xT")
                for kb in range(KB):
                    nc.sync.dma_start(out=xT_sb[:, kb, :], in_=xT[kb * 128:(kb + 1) * 128, :])
                wqk_sb = sb_x.tile([128, KB, 2 * HPC * DH], f32r, tag="wqk")
                nc.sync.dma_start(out=wqk_sb[:], in_=wqk[:, :].rearrange("(kb p) m -> p kb m", p=128))
                wv_sb = sb_x.tile([128, KB, HPC * DH], f32r, tag="wv")
                nc.sync.dma_start(out=wv_sb[:], in_=wv[:, :].rearrange("(kb p) m -> p kb m", p=128))

                # qkT: mb 0 = q heads 01 | mb 1 = q heads 23 | mb 2 = k heads 01 | mb 3 = k heads 23
                for mb in [0, 2, 1, 3]:
                    for ich in range(NCH):
                        mps = ps_big.tile([128, 512], f32, tag="big", name=f"mps{mb}_{ich}")
                        for kb in range(KB):
                            nc.tensor.matmul(
                                mps[:],
                                wqk_sb[:, kb, mb * 128:(mb + 1) * 128],
                                xT_sb[:, kb, ich * 512:(ich + 1) * 512],
                                start=(kb == 0), stop=(kb == KB - 1))
                        nc.vector.tensor_copy(qk_sb[:, mb, ich * 512:(ich + 1) * 512], mps[:])

                # v natural; ones columns = whatever the evictions don't overwrite
                nc.vector.memset(v_sb[:], 1.0)
                for jb in range(JB):
                    vps = ps_big.tile([128, 256], f32, tag="big", name=f"vps{jb}")
                    for kb in range(KB):
                        nc.tensor.matmul(
                            vps[:],
                            xT_sb[:, kb, jb * 128:(jb + 1) * 128],
                            wv_sb[:, kb, :],
                            start=(kb == 0), stop=(kb == KB - 1))
                    nc.vector.tensor_copy(
                        v_sb[:, jb, :].rearrange("p (h c) -> p h c", c=VW)[:, :, 0:DH],
                        vps[:].rearrange("p (h c) -> p h c", c=DH))

            # ---- attention (bf16, 1024-wide query chunks) ----
            sb_attn = ctx.enter_context(tc.tile_pool(name="sb_attn", bufs=10))
            sb_work = ctx.enter_context(tc.tile_pool(name="sb_work", bufs=2))
            otmp_pool = ctx.enter_context(tc.tile_pool(name="otmp", bufs=3))
            rbc_pool = ctx.enter_context(tc.tile_pool(name="rbc", bufs=2))

            y_part = dram.tile([N, D], bf16, tag="y_part")
            y_red = dram.tile([NCH, 128, D], bf16, tag="y_red")
            groups = [[0, 1, 2, 3], [4, 5, 6, 7]]

            from collections import deque
            fillers = deque()

            def emit_proj_tile(ib, ec):
                ibs = slice(ib * 128, (ib + 1) * 128)
                yps = ps_big.tile([128, 512], f32, tag="big", name=f"yps{ib}_{ec}")
                for h in range(HPC):
                    nc.tensor.matmul(
                        yps[:],
                        o_sb[:, h, ibs],
                        wo_sb[:, h, ec * 512:(ec + 1) * 512],
                        start=(h == 0), stop=(h == HPC - 1))
                ysb = sb_work.tile([128, 512], bf16, tag="y", name=f"ysb{ib}_{ec}")
                with nc.allow_low_precision(reason="bf16 partials for the reduce-scatter"):
                    nc.vector.tensor_add(ysb[:], yps[:], bias_bc[:, ec * 512:(ec + 1) * 512])
                nc.sync.dma_start(out=y_part[ibs, ec * 512:(ec + 1) * 512], in_=ysb[:])

            def emit_rs(ch):
                nc.gpsimd.collective_compute(
                    "ReduceScatter",
                    mybir.AluOpType.add,
                    replica_groups=groups,
                    ins=[y_part[ch * 512:(ch + 1) * 512, :]],
                    outs=[y_red[ch]],
                )

            def push_proj_chunk(ch):
                for ib in range(ch * 4, (ch + 1) * 4):
                    for ec in range(D // 512):
                        fillers.append(lambda ib=ib, ec=ec: emit_proj_tile(ib, ec))
                fillers.append(lambda ch=ch: emit_rs(ch))

            def pop_fillers(k):
                for _ in range(min(k, len(fillers))):
                    fillers.popleft()()

            GRP = 2
            for iq in range(IQ):
                isl = slice(iq * 1024, (iq + 1) * 1024)
                otmps = {}
                rdens = {}
                for pair in range(2):
                    ops = {}
                    for s in range(2):
                        h = pair * 2 + s
                        ops[h] = ps_o.tile([65, 1024], f32, tag="o", name=f"ops{h}_{iq}")
                    ats = {}
                    AVB = 2  # exp groups per AV batch
                    NG = 2 * JB // GRP
                    for g in range(NG):
                        for s in range(2):
                            h = pair * 2 + s
                            psl = slice(s * 64, s * 64 + 64)
                            st = ps_big.tile([128, 1024], f32, tag="big", name=f"st{h}_{iq}_{g}")
                            for u in range(GRP):
                                jb = (g * GRP + u) % JB
                                ihalf = (g * GRP + u) // JB
                                nc.tensor.matmul(
                                    st[:, u * 512:(u + 1) * 512],
                                    qk_sb[psl, 2 + pair, jb * 128:(jb + 1) * 128],
                                    qk_sb[psl, pair, iq * 1024 + ihalf * 512:
                                          iq * 1024 + (ihalf + 1) * 512],
                                    start=True, stop=True)
                            at = sb_attn.tile([128, 1024], bf16, tag="attn", name=f"at{h}_{iq}_{g}")
                            nc.scalar.activation(at[:], st[:],
                                                 mybir.ActivationFunctionType.Exp,
                                                 scale=float(SCALE))
                            ats[h, g] = at
                        if g % AVB == AVB - 1:
                            for s in range(2):
                                h = pair * 2 + s
                                for gg in range(g - AVB + 1, g + 1):
                                    for u in range(GRP):
                                        jb = (gg * GRP + u) % JB
                                        ihalf = (gg * GRP + u) // JB
                                        nc.tensor.matmul(
                                            ops[h][:, ihalf * 512:(ihalf + 1) * 512],
                                            v_sb[:, jb, h * VW:(h + 1) * VW],
                                            ats[h, gg][:, u * 512:(u + 1) * 512],
                                            start=(jb == 0), stop=(jb == JB - 1))
                            pop_fillers(1)
                    # denominators straight from PSUM, then evict + normalize
                    for s in range(2):
                        h = pair * 2 + s
                        rden = sb_work.tile([1, 1024], f32r, tag="rden", name=f"rden{h}_{iq}")
                        with nc.allow_low_precision(reason="f32r rounding of softmax denom recip"):
                            nc.vector.reciprocal(rden[:], ops[h][64:65, :])
                        rdens[h] = rden
                        ot = otmp_pool.tile([64, 1024], f32, tag="otmp", name=f"otmp{h}_{iq}")
                        nc.vector.tensor_copy(ot[:], ops[h][0:64, :])
                        otmps[h] = ot
                for h in range(HPC):
                    rbc = rbc_pool.tile([128, 1024], f32, tag="rbc", name=f"rbc{h}_{iq}")
                    for u in range(2):
                        rps = ps_big.tile([128, 512], f32, tag="big", name=f"rps{h}_{iq}_{u}")
                        nc.tensor.matmul(rps[:], ones_sb[:], rdens[h][:, u * 512:(u + 1) * 512],
                                         start=True, stop=True)
                        nc.vector.tensor_copy(rbc[:, u * 512:(u + 1) * 512], rps[:])
                    nc.vector.tensor_mul(o_sb[:, h, isl], otmps[h][:], rbc[0:64, :])
                push_proj_chunk(iq * 2)
                push_proj_chunk(iq * 2 + 1)
            while fillers:
                fillers.popleft()()

            # ship the shards: bf16 -> f32 via SBUF (after all collectives)
            for ch in range(NCH):
                shb = sb_work.tile([128, D], bf16, tag="shb", name=f"shb{ch}")
                nc.gpsimd.dma_start(out=shb[:], in_=y_red[ch])
                shf = sb_work.tile([128, D], f32, tag="shf", name=f"shf{ch}")
                nc.vector.tensor_copy(shf[:], shb[:])
                nc.gpsimd.dma_start(out=y_out[ch * 128:(ch + 1) * 128, :], in_=shf[:])

    nc.finalize()
    return nc


def _make_in_maps(x, w_qkv, w_out, b_out):
    x = np.asarray(x, dtype=np.float32)
    w_qkv = np.asarray(w_qkv, dtype=np.float32)
    w_out = np.asarray(w_out, dtype=np.float32)
    b_out = np.asarray(b_out, dtype=np.float32)
    ones1 = np.ones((1, 128), dtype=np.float32)
    quart = np.full((1, 128), 0.25, dtype=np.float32)
    in_maps = []
    for c in range(NCORES):
        b = c // GSIZE
        h0 = (c % GSIZE) * HPC
        cols = np.arange(h0 * DH, (h0 + HPC) * DH)
        wq = w_qkv[:, cols]
        wk = w_qkv[:, D + cols]
        wv = w_qkv[:, 2 * D + cols]
        in_maps.append({
            "xT": np.ascontiguousarray(x[b].T),
            "wqk": np.ascontiguousarray(np.concatenate([wq, wk], axis=1)),
            "wv": np.ascontiguousarray(wv),
            "wout": np.ascontiguousarray(w_out[cols, :]),
            "bias": b_out[None, :],
            "ones1": ones1,
            "quart": quart,
        })
    return in_maps


def _assemble(results, x_shape):
    B = x_shape[0]
    y = np.empty((B, N, D), dtype=np.float32)
    for b in range(B):
        for g in range(GSIZE):
            shard = results[b * GSIZE + g]["y"]  # [NCH*128, D]
            for ch in range(NCH):
                y[b, ch * 512 + g * 128: ch * 512 + (g + 1) * 128, :] = \
                    shard[ch * 128:(ch + 1) * 128, :]
    return y


def kernel(x, w_qkv, w_out, b_out):
    from concourse.bass_utils import run_bass_kernel_spmd

    if "nc" not in _cached:
        _cached["nc"] = _build_nc()
    nc = _cached["nc"]
    in_maps = _make_in_maps(x, w_qkv, w_out, b_out)
    res = run_bass_kernel_spmd(nc, in_maps, list(range(NCORES)))
    return _assemble(res.results, np.asarray(x).shape)


# revision 12
# speedup vs baseline: 1.1788x; 1.1788x over previous
"""Multi-head attention on 8 Trainium2 NeuronCores.

Problem: x[2, 2048, 1024] -> qkv proj (w_qkv [1024, 3072], 16 heads x 64) ->
softmax attention -> out proj (w_out [1024, 1024] + b_out).

Sharding: core c in 0..7 handles batch b = c // 4 and heads 4*(c%4) .. 4*(c%4)+3.
Each core computes a partial output projection over its 4 heads' slice; the four
cores of each batch group ReduceScatter(add) the partials chunk-by-chunk in bf16
(bias/4 folded in on every core), overlapped with later attention chunks. Core
g of a group ends up with rows [ch*512 + g*128, +128) of each chunk; the host
reassembles the full output from all 8 cores' shards.

Per-core dataflow:
  qkT [512, 2048] = wqk.T @ xT        (fp32r matmuls; Q/K stored bf16,
                                       d-on-partitions layout)
  v   [2048, 260] = xT.T @ wv         (bf16, natural layout, +ones col per head)
  then per query-chunk iq (1024 rows), per head pair (bf16 matmuls, N=1024
  streams; the K=64 head pair shares the PE array via tile rows 0-63/64-127):
    S_T [j, i] = kT.T-slices @ qT
    attn_T = exp(S_T * 0.125)         (ScalarE, scale folded into activation)
    O_T[h] [65, 1024] = v_aug.T @ attn_T  (row 64 = softmax denominator, free)
    o[h] = O_T[0:64] * (1/denom bcast)    (PE K=1 broadcast + DVE multiply)
  y[ch] += o[h].T @ wout[h] (+bias/4)     (fp32r, K=64 per head, PSUM-accum)
  ReduceScatter(y[ch]) in bf16 over the 4-core batch group.
"""

import numpy as np

N = 2048          # sequence length per batch
D = 1024          # model dim
DH = 64           # head dim
HPC = 4           # heads per core
NCORES = 8
GSIZE = 4         # cores per reduce group
SCALE = DH ** -0.5
NCH = N // 512    # output projection / collective chunks

_cached = {}


def _build_nc():
    from contextlib import ExitStack

    import concourse.bacc as bacc
    import concourse.mybir as mybir
    from concourse import tile

    f32 = mybir.dt.float32
    f32r = mybir.dt.float32r
    bf16 = mybir.dt.bfloat16

    nc = bacc.Bacc(num_devices=NCORES)

    xT = nc.declare_dram_parameter("xT", [D, N], f32r, isOutput=False)
    wqk = nc.declare_dram_parameter("wqk", [D, 2 * HPC * DH], f32r, isOutput=False)
    wv = nc.declare_dram_parameter("wv", [D, HPC * DH], f32r, isOutput=False)
    wout = nc.declare_dram_parameter("wout", [HPC * DH, D], f32r, isOutput=False)
    bias = nc.declare_dram_parameter("bias", [1, D], f32r, isOutput=False)
    ones1 = nc.declare_dram_parameter("ones1", [1, 128], f32r, isOutput=False)
    quart = nc.declare_dram_parameter("quart", [1, 128], f32r, isOutput=False)
    # per-core output: NCH shards of 128 rows (this core's ReduceScatter slices)
    y_out = nc.declare_dram_parameter("y", [NCH * 128, D], f32, isOutput=True)

    KB = D // 128           # 8 contraction blocks for the projections
    JB = N // 128           # 16 key blocks
    IQ = N // 1024          # attention query chunks (1024 wide)
    VW = DH + 1             # v columns per head incl. ones column

    with tile.TileContext(nc) as tc:
        ctx = ExitStack()
        with ctx:
            sb = ctx.enter_context(tc.tile_pool(name="sb", bufs=1))
            ps_big = ctx.enter_context(tc.tile_pool(name="ps_big", bufs=3, space="PSUM"))
            ps_o = ctx.enter_context(tc.tile_pool(name="ps_o", bufs=2, space="PSUM"))
            dram = ctx.enter_context(tc.tile_pool(name="dram", bufs=1, space="DRAM"))

            # persistent SBUF residents
            qk_sb = sb.tile([128, 4, N], bf16, tag="qk")
            v_sb = sb.tile([128, JB, HPC * VW], bf16, tag="v")
            o_sb = sb.tile([64, HPC, N], f32r, tag="o")
            wo_sb = sb.tile([64, HPC, D], f32r, tag="wo")
            bias_bc = sb.tile([128, D], f32, tag="bias_bc")
            bias_sb = sb.tile([1, D], f32r, tag="bias")
            ones_sb = sb.tile([1, 128], f32r, tag="ones1")
            quart_sb = sb.tile([1, 128], f32r, tag="quart")

            nc.sync.dma_start(out=bias_sb[:], in_=bias[:, :])
            nc.sync.dma_start(out=ones_sb[:], in_=ones1[:, :])
            nc.sync.dma_start(out=quart_sb[:], in_=quart[:, :])
            for h in range(HPC):
                nc.sync.dma_start(out=wo_sb[:, h, :], in_=wout[h * DH:(h + 1) * DH, :])

            # bias/4 broadcast to 128 partitions (PE K=1 matmul)
            for ch in range(D // 512):
                bps = ps_big.tile([128, 512], f32, tag="big", name=f"bps{ch}")
                nc.tensor.matmul(bps[:], quart_sb[:], bias_sb[:, ch * 512:(ch + 1) * 512],
                                 start=True, stop=True)
                nc.vector.tensor_copy(bias_bc[:, ch * 512:(ch + 1) * 512], bps[:])

            # ---- stage 1: projections (xT + weights live only here) ----
            with ExitStack() as s1:
                sb_x = s1.enter_context(tc.tile_pool(name="sb_x", bufs=1))
                xT_sb = sb_x.tile([128, KB, N], f32r, tag="xT")
                for kb in range(KB):
                    for q in range(4):
                        nc.sync.dma_start(
                            out=xT_sb[:, kb, q * 512:(q + 1) * 512],
                            in_=xT[kb * 128:(kb + 1) * 128, q * 512:(q + 1) * 512])
                wqk_sb = sb_x.tile([128, KB, 2 * HPC * DH], f32r, tag="wqk")
                wv_sb = sb_x.tile([128, KB, HPC * DH], f32r, tag="wv")
                for kb in range(KB):
                    nc.sync.dma_start(out=wqk_sb[:, kb, :], in_=wqk[kb * 128:(kb + 1) * 128, :])
                    nc.sync.dma_start(out=wv_sb[:, kb, :], in_=wv[kb * 128:(kb + 1) * 128, :])

                # qkT: mb 0 = q heads 01 | mb 1 = q heads 23 | mb 2 = k heads 01 | mb 3 = k heads 23
                for mb in [0, 2, 1, 3]:
                    for ich in range(NCH):
                        mps = ps_big.tile([128, 512], f32, tag="big", name=f"mps{mb}_{ich}")
                        for kb in range(KB):
                            nc.tensor.matmul(
                                mps[:],
                                wqk_sb[:, kb, mb * 128:(mb + 1) * 128],
                                xT_sb[:, kb, ich * 512:(ich + 1) * 512],
                                start=(kb == 0), stop=(kb == KB - 1))
                        nc.vector.tensor_copy(qk_sb[:, mb, ich * 512:(ich + 1) * 512], mps[:])

                # v natural; ones columns = whatever the evictions don't overwrite
                nc.vector.memset(v_sb[:], 1.0)
                for jb in range(JB):
                    vps = ps_big.tile([128, 256], f32, tag="big", name=f"vps{jb}")
                    for kb in range(KB):
                        nc.tensor.matmul(
                            vps[:],
                            xT_sb[:, kb, jb * 128:(jb + 1) * 128],
                            wv_sb[:, kb, :],
                            start=(kb == 0), stop=(kb == KB - 1))
                    nc.vector.tensor_copy(
                        v_sb[:, jb, :].rearrange("p (h c) -> p h c", c=VW)[:, :, 0:DH],
                        vps[:].rearrange("p (h c) -> p h c", c=DH))

            # ---- attention (bf16, 1024-wide query chunks) ----
            sb_attn = ctx.enter_context(tc.tile_pool(name="sb_attn", bufs=10))
            sb_work = ctx.enter_context(tc.tile_pool(name="sb_work", bufs=2))
            otmp_pool = ctx.enter_context(tc.tile_pool(name="otmp", bufs=3))
            rbc_pool = ctx.enter_context(tc.tile_pool(name="rbc", bufs=2))

            y_part = dram.tile([N, D], bf16, tag="y_part")
            y_red = dram.tile([NCH, 128, D], bf16, tag="y_red")
            groups = [[0, 1, 2, 3], [4, 5, 6, 7]]

            def proj_chunk(ch):
                """output projection rows [ch*512, +512) + bias/4, DMA, RS"""
                for ib in range(ch * 4, (ch + 1) * 4):
                    ibs = slice(ib * 128, (ib + 1) * 128)
                    for ec in range(D // 512):
                        yps = ps_big.tile([128, 512], f32, tag="big", name=f"yps{ib}_{ec}")
                        for h in range(HPC):
                            nc.tensor.matmul(
                                yps[:],
                                o_sb[:, h, ibs],
                                wo_sb[:, h, ec * 512:(ec + 1) * 512],
                                start=(h == 0), stop=(h == HPC - 1))
                        ysb = sb_work.tile([128, 512], bf16, tag="y", name=f"ysb{ib}_{ec}")
                        with nc.allow_low_precision(reason="bf16 partials for the reduce-scatter"):
                            nc.vector.tensor_add(ysb[:], yps[:], bias_bc[:, ec * 512:(ec + 1) * 512])
                        nc.sync.dma_start(out=y_part[ibs, ec * 512:(ec + 1) * 512], in_=ysb[:])
                nc.gpsimd.collective_compute(
                    "ReduceScatter",
                    mybir.AluOpType.add,
                    replica_groups=groups,
                    ins=[y_part[ch * 512:(ch + 1) * 512, :]],
                    outs=[y_red[ch]],
                )

            GRP = 2
            for ich in range(NCH):
                isl = slice(ich * 512, (ich + 1) * 512)
                otmps = {}
                rdens = {}
                for pair in range(2):
                    ops = {}
                    for s in range(2):
                        h = pair * 2 + s
                        ops[h] = ps_o.tile([65, 512], f32, tag="o", name=f"ops{h}_{ich}")
                    ats = {}
                    AVB = 2  # exp groups per AV batch
                    NG = JB // GRP
                    for g in range(NG):
                        for s in range(2):
                            h = pair * 2 + s
                            psl = slice(s * 64, s * 64 + 64)
                            st = ps_big.tile([128, 1024], f32, tag="big", name=f"st{h}_{ich}_{g}")
                            for u in range(GRP):
                                jb = g * GRP + u
                                nc.tensor.matmul(
                                    st[:, u * 512:(u + 1) * 512],
                                    qk_sb[psl, 2 + pair, jb * 128:(jb + 1) * 128],
                                    qk_sb[psl, pair, isl],
                                    start=True, stop=True)
                            at = sb_attn.tile([128, 1024], bf16, tag="attn", name=f"at{h}_{ich}_{g}")
                            nc.scalar.activation(at[:], st[:],
                                                 mybir.ActivationFunctionType.Exp,
                                                 scale=float(SCALE))
                            ats[h, g] = at
                        if g % AVB == AVB - 1:
                            for s in range(2):
                                h = pair * 2 + s
                                for gg in range(g - AVB + 1, g + 1):
                                    for u in range(GRP):
                                        jb = gg * GRP + u
                                        nc.tensor.matmul(
                                            ops[h][:],
                                            v_sb[:, jb, h * VW:(h + 1) * VW],
                                            ats[h, gg][:, u * 512:(u + 1) * 512],
                                            start=(jb == 0), stop=(jb == JB - 1))
                    # denominators straight from PSUM, then evict
                    for s in range(2):
                        h = pair * 2 + s
                        rden = sb_work.tile([1, 512], f32r, tag="rden", name=f"rden{h}_{ich}")
                        with nc.allow_low_precision(reason="f32r rounding of softmax denom recip"):
                            nc.vector.reciprocal(rden[:], ops[h][64:65, :])
                        rdens[h] = rden
                        ot = otmp_pool.tile([64, 512], f32, tag="otmp", name=f"otmp{h}_{ich}")
                        nc.vector.tensor_copy(ot[:], ops[h][0:64, :])
                        otmps[h] = ot
                for h in range(HPC):
                    rbc = rbc_pool.tile([128, 512], f32, tag="rbc", name=f"rbc{h}_{ich}")
                    rps = ps_big.tile([128, 512], f32, tag="big", name=f"rps{h}_{ich}")
                    nc.tensor.matmul(rps[:], ones_sb[:], rdens[h][:], start=True, stop=True)
                    nc.vector.tensor_copy(rbc[:], rps[:])
                    nc.vector.tensor_mul(o_sb[:, h, isl], otmps[h][:], rbc[0:64, :])
                proj_chunk(ich)

            # ship the shards: bf16 -> f32 via SBUF (after all collectives)
            for ch in range(NCH):
                shb = sb_work.tile([128, D], bf16, tag="shb", name=f"shb{ch}")
                nc.gpsimd.dma_start(out=shb[:], in_=y_red[ch])
                shf = sb_work.tile([128, D], f32, tag="shf", name=f"shf{ch}")
                nc.vector.tensor_copy(shf[:], shb[:])
                nc.gpsimd.dma_start(out=y_out[ch * 128:(ch + 1) * 128, :], in_=shf[:])

    nc.finalize()
    return nc


def _make_in_maps(x, w_qkv, w_out, b_out):
    x = np.asarray(x, dtype=np.float32)
    w_qkv = np.asarray(w_qkv, dtype=np.float32)
    w_out = np.asarray(w_out, dtype=np.float32)
    b_out = np.asarray(b_out, dtype=np.float32)
    ones1 = np.ones((1, 128), dtype=np.float32)
    quart = np.full((1, 128), 0.25, dtype=np.float32)
    in_maps = []
    for c in range(NCORES):
        b = c // GSIZE
        h0 = (c % GSIZE) * HPC
        cols = np.arange(h0 * DH, (h0 + HPC) * DH)
        wq = w_qkv[:, cols]
        wk = w_qkv[:, D + cols]
        wv = w_qkv[:, 2 * D + cols]
        in_maps.append({
            "xT": np.ascontiguousarray(x[b].T),
            "wqk": np.ascontiguousarray(np.concatenate([wq, wk], axis=1)),
            "wv": np.ascontiguousarray(wv),
            "wout": np.ascontiguousarray(w_out[cols, :]),
            "bias": b_out[None, :],
            "ones1": ones1,
            "quart": quart,
        })
    return in_maps


def _assemble(results, x_shape):
    B = x_shape[0]
    y = np.empty((B, N, D), dtype=np.float32)
    for b in range(B):
        for g in range(GSIZE):
            shard = results[b * GSIZE + g]["y"]  # [NCH*128, D]
            for ch in range(NCH):
                y[b, ch * 512 + g * 128: ch * 512 + (g + 1) * 128, :] = \
                    shard[ch * 128:(ch + 1) * 128, :]
    return y


def kernel(x, w_qkv, w_out, b_out):
    from concourse.bass_utils import run_bass_kernel_spmd

    if "nc" not in _cached:
        _cached["nc"] = _build_nc()
    nc = _cached["nc"]
    in_maps = _make_in_maps(x, w_qkv, w_out, b_out)
    res = run_bass_kernel_spmd(nc, in_maps, list(range(NCORES)))
    return _assemble(res.results, np.asarray(x).shape)
